# revision 1
# baseline (speedup 1.0000x reference)
"""Chebyshev graph-conv (gnn_message_passing) Trainium2 kernel.

Reference computation (see problem):
    x0 = inputs [1,8,V,8,8,8] -> [V, Fin*B*X*Y*Z]
    Chebyshev recurrence with sparse Laplacian (COO, 8 entries/row), K=5
    out = einsum('kvfbxyz,kfo->bovxyz', cheb, weight) + bias

Sharding: dense dim D = Fin*XYZ split over the XYZ axis across 8 cores
(64 spatial positions per core -> local D = 64*8 = 512, laid out d = s*8+f).

Per-core algorithm (all compute on device):
  - spmv via SWDGE dma_gather of x rows from HBM (indices are runtime data)
    followed by PE selection-matmuls that fold the vals in and do the
    8-way segment sum into PSUM.
  - Chebyshev combine (x_next = 2*psum - x_prev) on DVE.
  - cheb terms transposed with PE transpose-mode; output einsum over (k,f)
    runs as PE matmuls with block-diagonal weight-selection matrices,
    accumulated into SBUF (bias folded into the k=0 pass).
"""

import sys

for _p in ("/opt/trn_rl_repo", "/root/.axon_site/_ro/trn_rl_repo"):
    if _p not in sys.path:
        sys.path.append(_p)

import numpy as np

V = 2562
DEG = 8
B, FIN, FOUT, K = 1, 8, 16, 5
XYZ = 512
NCORES = 8
SLOC = XYZ // NCORES  # 64 spatial positions per core
D = SLOC * FIN  # 512 local dense dim, d = s_loc*8 + f

VP = 2688  # V padded to 21*128
NT = VP // 128  # 21 v-tiles
EPAD = VP * DEG  # 21504 padded edges
NCHUNK = EPAD // 128  # 168 edge chunks of 128 (16 v's each)
NVCH = 6  # v-chunks of up to 4 v-tiles (5*4 + 1)

_COMPILED = [None]
LAST_RESULT = [None]


def _build(gch=8):
    import os as _os

    _skip_out = bool(int(_os.environ.get("SKIP_OUT", "0")))
    _skip_t = bool(int(_os.environ.get("SKIP_T", "0")))
    from contextlib import ExitStack

    import concourse.mybir as mybir
    import concourse.tile as tile
    from concourse import bacc

    fp32 = mybir.dt.float32
    i16 = mybir.dt.int16
    Alu = mybir.AluOpType

    nc = bacc.Bacc(None, target_bir_lowering=False)

    bf16 = mybir.dt.bfloat16
    f32r = mybir.dt.float32r
    x0g = nc.dram_tensor("x0g", [VP, D], bf16, kind="ExternalInput")
    idxd = nc.dram_tensor("idx", [128, NT * gch * 8], i16, kind="ExternalInput")
    seld = nc.dram_tensor("sel", [128, NT * gch, 128], bf16, kind="ExternalInput")
    wseld = nc.dram_tensor("wsel", [128, K * 2, 128], fp32, kind="ExternalInput")
    biasd = nc.dram_tensor("biasx", [128, 2], fp32, kind="ExternalInput")
    outd = nc.dram_tensor("outT", [8, 128, VP], fp32, kind="ExternalOutput")

    from concourse.masks import make_identity

    with ExitStack() as ctx:
        tc = ctx.enter_context(tile.TileContext(nc))
        const = ctx.enter_context(tc.tile_pool(name="const", bufs=1))
        accp = ctx.enter_context(tc.tile_pool(name="acc", bufs=1))
        dram = ctx.enter_context(tc.tile_pool(name="dram", bufs=1, space="DRAM"))
        gp = ctx.enter_context(tc.tile_pool(name="g", bufs=4))
        sp = ctx.enter_context(tc.tile_pool(name="stream", bufs=4))
        xtp = ctx.enter_context(tc.tile_pool(name="xt", bufs=3))
        pp = ctx.enter_context(tc.tile_pool(name="psum", bufs=2, space="PSUM"))
        ppv = ctx.enter_context(tc.tile_pool(name="psumv", bufs=3, space="PSUM"))

        SEL = const.tile([128, NT * gch, 128], bf16)
        WSEL = const.tile([128, K * 2, 128], f32r)
        IDX = const.tile([128, NT * gch * 8], i16)
        BIA = const.tile([128, 2], fp32)
        IDN = const.tile([128, 128], fp32)
        IDNB = const.tile([128, 128], bf16)
        nq = NT * gch
        for q in range(4):
            qs = slice(q * nq // 4, (q + 1) * nq // 4)
            nc.sync.dma_start(SEL[:, qs, :], seld[:, qs, :])
        WSELF = sp.tile([128, K * 2 * 128], fp32, tag="wself", name="WSELF", bufs=1)
        nc.sync.dma_start(WSELF[:], wseld[:].rearrange("p a b -> p (a b)"))
        nc.vector.tensor_copy(
            WSEL[:].rearrange("p a b -> p (a b)"), WSELF[:]
        )
        nc.sync.dma_start(IDX[:], idxd[:])
        nc.sync.dma_start(BIA[:], biasd[:])
        make_identity(nc, IDN[:])
        nc.vector.tensor_copy(IDNB[:], IDN[:])

        ACC = [accp.tile([128, VP], fp32, tag=f"acc{i}", name=f"acc{i}") for i in range(8)]
        if _skip_out or _skip_t:
            for i in range(8):
                nc.vector.memset(ACC[i][:], 0.0)
        xgb = [dram.tile([VP, D], bf16, tag=f"xgb{i}", name=f"xgb{i}") for i in range(3)]

        def transpose_to(xv, xTt, vl, dt=fp32):
            """xv [128 v, 512 d] -> xTt[:, t, 128*vl:+128] for t in 0..3."""
            if _skip_t:
                return
            psT = pp.tile([128, 512], dt, tag="psT")
            for t in range(4):
                s = slice(128 * t, 128 * (t + 1))
                nc.tensor.transpose(
                    psT[:, s], xv[:, s], IDN[:] if dt == fp32 else IDNB[:]
                )
            for t in range(4):
                nc.scalar.copy(
                    xTt[:, t, 128 * vl : 128 * (vl + 1)],
                    psT[:, 128 * t : 128 * (t + 1)],
                )

        def out_stage(k, ch, nvt, xTt):
            """ACC[(t,h)][:, chunk] (+)= WSEL_k,h^T @ xT_t  (+bias at k=0)."""
            if _skip_out or _skip_t:
                return
            n = nvt * 128
            cs = slice(512 * ch, 512 * ch + n)
            for t in range(4):
                for h in range(2):
                    i = t * 2 + h
                    psO = pp.tile([128, 512], fp32, tag="psO")
                    nc.tensor.matmul(
                        psO[:, :n],
                        WSEL[:, k * 2 + h, :],
                        xTt[:, t, :n],
                        start=True,
                        stop=True,
                    )
                    if k == 0:
                        nc.vector.tensor_scalar(
                            ACC[i][:, cs], psO[:, :n], BIA[:, h : h + 1], None, Alu.add
                        )
                    else:
                        nc.vector.tensor_tensor(
                            ACC[i][:, cs], ACC[i][:, cs], psO[:, :n], Alu.add
                        )
                    if k == K - 1:
                        nc.sync.dma_start(outd[i, :, cs], ACC[i][:, cs])

        # ---- k = 0: cheb_0 = x0 ----
        for ch in range(NVCH):
            nvt = 4 if ch < 5 else 1
            xTt = xtp.tile([128, 4, 512], f32r, tag="xTt")
            for vl in range(nvt):
                vt = 4 * ch + vl
                xv0 = sp.tile([128, D], bf16, tag="xv0", bufs=2)
                nc.sync.dma_start(xv0[:], x0g[128 * vt : 128 * (vt + 1), :])
                transpose_to(xv0, xTt, vl, dt=bf16)
            out_stage(0, ch, nvt, xTt)

        # ---- k = 1..4: x_k = 2 L x_{k-1} - x_{k-2}   (k=1: x_1 = L x_0) ----
        for k in range(1, K):
            src = x0g if k == 1 else xgb[(k - 2) % 3]
            prev = None if k == 1 else (x0g if k == 2 else xgb[(k - 3) % 3])
            dstb = xgb[(k - 1) % 3]
            for ch in range(NVCH):
                nvt = 4 if ch < 5 else 1
                xTt = xtp.tile([128, 4, 512], f32r, tag="xTt")
                for vl in range(nvt):
                    vt = 4 * ch + vl
                    psV = ppv.tile([128, 512], fp32, tag="psV")
                    g = gp.tile([128, gch, D], bf16, tag="g")
                    with tc.high_priority(offset=400):
                        nc.gpsimd.dma_gather(
                            g[:],
                            src[:],
                            IDX[:, gch * 8 * vt : gch * 8 * (vt + 1)],
                            num_idxs=gch * 128,
                            num_idxs_reg=gch * 128,
                            elem_size=D,
                        )
                    for j in range(gch):
                        nc.tensor.matmul(
                            psV[:],
                            SEL[:, gch * vt + j, :],
                            g[:, j, :],
                            start=(j == 0),
                            stop=(j == gch - 1),
                        )
                    xv = sp.tile([128, D], fp32, tag="xv")
                    with tc.high_priority(offset=600):
                        if k == 1:
                            nc.vector.tensor_copy(xv[:], psV[:])
                        else:
                            pv = sp.tile([128, D], bf16, tag="pv", bufs=6)
                            with tc.high_priority(offset=1200):
                                nc.sync.dma_start(
                                    pv[:], prev[128 * vt : 128 * (vt + 1), :]
                                )
                            nc.vector.scalar_tensor_tensor(
                                xv[:], psV[:], 2.0, pv[:], Alu.mult, Alu.subtract
                            )
                        if k < K - 1:
                            xvb = sp.tile([128, D], bf16, tag="xvb")
                            nc.vector.tensor_copy(xvb[:], xv[:])
                            nc.sync.dma_start(
                                dstb[128 * vt : 128 * (vt + 1), :], xvb[:]
                            )
                    transpose_to(xv, xTt, vl)
                out_stage(k, ch, nvt, xTt)


    nc.compile()
    return nc


def _host_prep(inputs, lap_rows, lap_cols, lap_vals, weight, bias):
    inputs = np.asarray(inputs, dtype=np.float32)
    lap_rows = np.asarray(lap_rows)
    lap_cols = np.asarray(lap_cols)
    lap_vals = np.asarray(lap_vals, dtype=np.float32)
    weight = np.asarray(weight, dtype=np.float32)
    bias = np.asarray(bias, dtype=np.float32)

    nnz = lap_rows.shape[0]
    order = np.argsort(lap_rows, kind="stable")
    srows = lap_rows[order]
    assert np.array_equal(
        np.repeat(np.arange(V, dtype=srows.dtype), DEG), srows
    ), "expected exactly DEG entries per row"
    e_cols = np.zeros(EPAD, np.int64)
    e_vals = np.zeros(EPAD, np.float32)
    e_cols[:nnz] = lap_cols[order]
    e_vals[:nnz] = lap_vals[order]

    # per-v-tile dedup: gather each unique col once; SEL folds vals and
    # scatters every (unique col -> output v) pair of the tile
    uniq = []
    for vt in range(NT):
        ecols = e_cols[1024 * vt : 1024 * (vt + 1)]
        uniq.append(np.unique(ecols))
    gch = max(2, max((len(u) + 127) // 128 for u in uniq))

    idx_np = np.zeros((128, NT * gch * 8), np.int16)
    sel_np = np.zeros((128, NT * gch, 128), np.float32)
    for vt in range(NT):
        u = uniq[vt]
        slots = np.zeros(gch * 128, np.int64)
        slots[: len(u)] = u
        w = slots.reshape(gch * 8, 16).T.astype(np.int16)  # wrapped-16
        idx_np[:, gch * 8 * vt : gch * 8 * (vt + 1)] = np.tile(w, (8, 1))
        col2slot = np.zeros(V + 1, np.int64)
        col2slot[u] = np.arange(len(u))
        ecols = e_cols[1024 * vt : 1024 * (vt + 1)]
        evals = e_vals[1024 * vt : 1024 * (vt + 1)]
        sl = col2slot[ecols]
        m = np.arange(1024) // DEG  # output row within v-tile
        np.add.at(sel_np, (sl % 128, gch * vt + sl // 128, m), evals)
    import ml_dtypes

    sel_np = sel_np.astype(ml_dtypes.bfloat16)

    # output-stage weight selection: rows p=s_loc*8+f, cols q=s_loc*8+o
    wsel_np = np.zeros((128, K * 2, 128), np.float32)
    sl = np.arange(16)
    for k in range(K):
        for h in range(2):
            for f in range(FIN):
                for o in range(8):
                    wsel_np[sl * 8 + f, k * 2 + h, sl * 8 + o] = weight[k, f, 8 * h + o]

    bias_np = np.zeros((128, 2), np.float32)
    p = np.arange(128)
    for h in range(2):
        bias_np[p, h] = bias[8 * h + p % 8]

    # x0 shards: [V, s, f] per core
    xt = inputs.reshape(FIN, V, XYZ).transpose(1, 2, 0)  # [V, 512, 8]
    x0s = []
    for m in range(NCORES):
        x0m = np.zeros((VP, D), np.float32)
        x0m[:V] = xt[:, SLOC * m : SLOC * (m + 1), :].reshape(V, D)
        x0s.append(x0m)
    return x0s, idx_np, sel_np, wsel_np, bias_np


def kernel(inputs, lap_rows, lap_cols, lap_vals, weight, bias):
    import ml_dtypes as _ml

    from concourse.bass_utils import run_bass_kernel_spmd

    x0s, idx_np, sel_np, wsel_np, bias_np = _host_prep(
        inputs, lap_rows, lap_cols, lap_vals, weight, bias
    )

    gch = idx_np.shape[1] // (NT * 8)
    if _COMPILED[0] is None or _COMPILED[0][0] != gch:
        _COMPILED[0] = (gch, _build(gch))
    nc = _COMPILED[0][1]

    in_maps = [
        {
            "x0g": x0s[m].astype(_ml.bfloat16),
            "idx": idx_np,
            "sel": sel_np,
            "wsel": wsel_np,
            "biasx": bias_np,
        }
        for m in range(NCORES)
    ]
    import os

    trace = bool(int(os.environ.get("KERNEL_TRACE", "0")))
    res = run_bass_kernel_spmd(
        nc, in_maps, core_ids=list(range(NCORES)), trace=trace
    )
    LAST_RESULT[0] = res

    # unshard: outT [8=(t,h), 128=(s_loc,o_loc), VP] per core
    parts = []
    for m in range(NCORES):
        r = res.results[m]["outT"]  # [8, 128, VP]
        r = r.reshape(4, 2, 16, 8, VP)[:, :, :, :, :V]  # [t, h, sl, ol, v]
        # o = 8h + ol ; s_local_in_core = 16t + sl
        r = r.transpose(1, 3, 4, 0, 2).reshape(FOUT, V, SLOC)  # [o, v, s]
        parts.append(r)
    out = np.concatenate(parts, axis=2)  # [o, v, 512]
    return np.ascontiguousarray(
        out.reshape(1, FOUT, V, 8, 8, 8).astype(np.float32)
    )



# revision 12
# speedup vs baseline: 1.1247x; 1.1247x over previous
"""Chebyshev graph-conv (gnn_message_passing) Trainium2 kernel.

Reference computation:
    x0 = inputs [1,8,V,8,8,8] -> [V, Fin*B*X*Y*Z]
    Chebyshev recurrence with sparse Laplacian (COO, 8 entries/row), K=5
    out = einsum('kvfbxyz,kfo->bovxyz', cheb, weight) + bias

Sharding: dense dim D = Fin*XYZ split over the XYZ axis across 8 cores
(64 spatial positions per core -> local D = 64*8 = 512, laid out d = s*8+f).

Per-core algorithm (v2):
  - spmv: per-v-tile deduped SWDGE dma_gather of x rows in float8_e3m4
    (halves gather HBM bytes vs bf16; E3M4's 4 mantissa bits keep the
    recurrence error ~1e-2 << 2e-2 budget), folded by PE matmuls with
    bf16 selection matrices into PSUM (8-way segment sum + edge values).
  - All 5 Chebyshev terms stay resident in SBUF (bf16); the combine
    (x_k = 2*psum - x_{k-2}) reads prev from SBUF, writes the bf16
    resident slice + an e3m4 DRAM copy (gather source for the next spmv).
  - Output einsum accumulates over k in PSUM (tensor engine) from
    PE-transposed resident cheb tiles; bias added on drain; out stored
    bf16 and upcast on host.
"""

import sys

for _p in ("/opt/trn_rl_repo", "/root/.axon_site/_ro/trn_rl_repo"):
    if _p not in sys.path:
        sys.path.append(_p)

import numpy as np

V = 2562
DEG = 8
B, FIN, FOUT, K = 1, 8, 16, 5
XYZ = 512
NCORES = 8
SLOC = XYZ // NCORES  # 64 spatial positions per core
D = SLOC * FIN  # 512 local dense dim, d = s_loc*8 + f

VP = 2688  # V padded to 21*128
NT = VP // 128  # 21 v-tiles
EPAD = VP * DEG  # 21504 padded edges
NVCH = 6  # v-chunks of up to 4 v-tiles (5*4 + 1)

_COMPILED = [None]
LAST_RESULT = [None]


def _build(gchs, choffs):
    """gchs[vt] = gathered 128-row groups for v-tile vt; choffs[ch] = group
    offset of chunk ch (choffs[NVCH] = total groups G)."""
    import os as _os
    from contextlib import ExitStack

    import concourse.mybir as mybir
    import concourse.tile as tile
    from concourse import bacc

    _skip_out = bool(int(_os.environ.get("SKIP_OUT", "0")))
    _skip_rec = bool(int(_os.environ.get("SKIP_REC", "0")))
    _xe_from_psv = bool(int(_os.environ.get("XE_FROM_PSV", "0")))
    _split_copy = bool(int(_os.environ.get("SPLIT_COPY", "0")))
    _gather_bf16 = bool(int(_os.environ.get("GATHER_BF16", "0")))
    _single_gather = bool(int(_os.environ.get("SINGLE_GATHER", "1")))
    _out_k0 = bool(int(_os.environ.get("OUT_K0", "0")))
    _dump_ch = int(_os.environ.get("DUMP_CH", "-1"))

    fp32 = mybir.dt.float32
    i16 = mybir.dt.int16
    bf16 = mybir.dt.bfloat16
    f8e3 = mybir.dt.float8e3
    Alu = mybir.AluOpType

    G = choffs[NVCH]
    gdt_is_bf16 = _gather_bf16
    gch_max = max(
        choffs[ch + 1] - choffs[ch] for ch in range(NVCH)
    )

    nc = bacc.Bacc(None, target_bir_lowering=False)

    x0b = nc.dram_tensor("x0b", [VP, D], bf16, kind="ExternalInput")
    gdt = bf16 if _gather_bf16 else f8e3
    x0e = nc.dram_tensor("x0e", [VP, D], gdt, kind="ExternalInput")
    idxd = nc.dram_tensor("idx", [128, G * 8], i16, kind="ExternalInput")
    seld = nc.dram_tensor("sel", [128, G, 128], bf16, kind="ExternalInput")
    wseld = nc.dram_tensor("wsel", [128, K * 2, 128], bf16, kind="ExternalInput")
    biasd = nc.dram_tensor("biasx", [128, 2], fp32, kind="ExternalInput")
    outd = nc.dram_tensor("outT", [8, 128, VP], bf16, kind="ExternalOutput")
    chdump = (
        nc.dram_tensor("chdump", [128, NT, D], bf16, kind="ExternalOutput")
        if _dump_ch >= 0
        else None
    )

    from concourse.masks import make_identity

    with ExitStack() as ctx:
        tc = ctx.enter_context(tile.TileContext(nc))
        const = ctx.enter_context(tc.tile_pool(name="const", bufs=1))
        chp = ctx.enter_context(tc.tile_pool(name="cheb", bufs=1))
        dram = ctx.enter_context(tc.tile_pool(name="dram", bufs=1, space="DRAM"))
        gp = ctx.enter_context(tc.tile_pool(name="g", bufs=2))
        sp = ctx.enter_context(tc.tile_pool(name="stream", bufs=4))
        xtp = ctx.enter_context(tc.tile_pool(name="xt", bufs=2))
        op = ctx.enter_context(tc.tile_pool(name="ob", bufs=2))
        ppv = ctx.enter_context(tc.tile_pool(name="psumv", bufs=2, space="PSUM"))
        ppo = ctx.enter_context(tc.tile_pool(name="psumo", bufs=1, space="PSUM"))
        ppt = ctx.enter_context(tc.tile_pool(name="psumt", bufs=2, space="PSUM"))

        SEL = const.tile([128, G, 128], bf16)
        WSEL = const.tile([128, K * 2, 128], bf16)
        IDX = const.tile([128, G * 8], i16)
        BIA = const.tile([128, 2], fp32)
        IDNB = const.tile([128, 128], bf16)
        IDN = const.tile([128, 128], fp32)
        for q in range(4):
            qs = slice(q * G // 4, (q + 1) * G // 4 if q < 3 else G)
            nc.sync.dma_start(SEL[:, qs, :], seld[:, qs, :])
        nc.sync.dma_start(WSEL[:], wseld[:])
        nc.sync.dma_start(IDX[:], idxd[:])
        nc.sync.dma_start(BIA[:], biasd[:])
        make_identity(nc, IDN[:])
        nc.vector.tensor_copy(IDNB[:], IDN[:])

        # resident Chebyshev terms, bf16 [128, NT*512]
        CH = [
            chp.tile([128, NT, D], bf16, tag=f"ch{k}", name=f"ch{k}")
            for k in range(K)
        ]
        xg = [dram.tile([VP, D], gdt, tag=f"xg{i}", name=f"xg{i}") for i in range(3)]

        def out_stage(ch, nvt, k_src):
            """PSUM-accumulated over k output for chunk ch (nvt v-tiles)."""
            if _skip_out:
                return
            n = nvt * 128
            for half in range(2):
                psO = ppo.tile([128, 4, 512], fp32, tag="psO")
                for k in range(K):
                    xT = xtp.tile([128, 2, 512], bf16, tag="xT")
                    for vl in range(nvt):
                        vt = 4 * ch + vl
                        psT = ppt.tile([128, 2, 128], bf16, tag="psT")
                        for ti in range(2):
                            t = 2 * half + ti
                            nc.tensor.transpose(
                                psT[:, ti, :],
                                k_src(k)[:, vt, 128 * t : 128 * (t + 1)],
                                IDNB[:],
                            )
                        if _split_copy:
                            for ti in range(2):
                                nc.scalar.copy(
                                    xT[:, ti, 128 * vl : 128 * (vl + 1)],
                                    psT[:, ti, :],
                                )
                        else:
                            nc.scalar.copy(
                                xT[:, :, 128 * vl : 128 * (vl + 1)], psT[:]
                            )
                    for ti in range(2):
                        for h in range(2):
                            nc.tensor.matmul(
                                psO[:, 2 * ti + h, :n],
                                WSEL[:, k * 2 + h, :],
                                xT[:, ti, :n],
                                start=(k == 0),
                                stop=(k == K - 1),
                            )
                ob = op.tile([128, 4, 512], bf16, tag="ob")
                for ti in range(2):
                    for h in range(2):
                        i = (2 * half + ti) * 2 + h
                        nc.vector.tensor_scalar(
                            ob[:, 2 * ti + h, :n],
                            psO[:, 2 * ti + h, :n],
                            BIA[:, h : h + 1],
                            None,
                            Alu.add,
                        )
                        nc.sync.dma_start(
                            outd[i, :, 512 * ch : 512 * ch + n],
                            ob[:, 2 * ti + h, :n],
                        )

        # ---- k = 0: cheb_0 = x0 (straight load into residency) ----
        for vt in range(NT):
            nc.sync.dma_start(CH[0][:, vt, :], x0b[128 * vt : 128 * (vt + 1), :])

        # ---- k = 1..4: x_k = 2 L x_{k-1} - x_{k-2}   (k=1: x_1 = L x_0) ----
        for k in range(1, (0 if _skip_rec else K)):
            src = x0e if k == 1 else xg[k - 2]
            for ch in range(NVCH):
                nvt = 4 if ch < 5 else 1
                gch_ch = choffs[ch + 1] - choffs[ch]
                g = gp.tile([128, gch_max, D], gdt, tag="g")
                if _single_gather:
                    goff0 = 0
                    for vl in range(nvt):
                        vt = 4 * ch + vl
                        nidx = gchs[vt] * 128
                        with tc.high_priority(offset=400):
                            nc.gpsimd.dma_gather(
                                g[:, goff0 : goff0 + gchs[vt], :],
                                src[:],
                                IDX[
                                    :,
                                    8 * (choffs[ch] + goff0) : 8
                                    * (choffs[ch] + goff0 + gchs[vt]),
                                ],
                                num_idxs=nidx,
                                num_idxs_reg=nidx,
                                elem_size=D,
                            )
                        goff0 += gchs[vt]
                else:
                    with tc.high_priority(offset=400):
                        nc.gpsimd.dma_gather(
                            g[:, :gch_ch, :],
                            src[:],
                            IDX[:, 8 * choffs[ch] : 8 * choffs[ch + 1]],
                            num_idxs=gch_ch * 128,
                            num_idxs_reg=gch_ch * 128,
                            elem_size=D,
                        )
                for vl in range(nvt):
                    vt = 4 * ch + vl
                    goff = None  # group offset of this tile within chunk
                    goff = sum(
                        gchs[4 * ch + j] for j in range(vl)
                    )
                    psV = ppv.tile([128, 512], fp32, tag="psV")
                    for j in range(gchs[vt]):
                        nc.tensor.matmul(
                            psV[:],
                            SEL[:, choffs[ch] + goff + j, :],
                            g[:, goff + j, :],
                            start=(j == 0),
                            stop=(j == gchs[vt] - 1),
                        )
                    with tc.high_priority(offset=600):
                        if k == 1:
                            nc.vector.tensor_copy(CH[1][:, vt, :], psV[:])
                        else:
                            nc.vector.scalar_tensor_tensor(
                                CH[k][:, vt, :],
                                psV[:],
                                2.0,
                                CH[k - 2][:, vt, :],
                                Alu.mult,
                                Alu.subtract,
                            )
                        if k < K - 1:
                            xe = sp.tile([128, D], gdt, tag="xe")
                            if _xe_from_psv:
                                nc.vector.tensor_copy(xe[:], psV[:])
                            else:
                                nc.vector.tensor_copy(xe[:], CH[k][:, vt, :])
                            nc.sync.dma_start(
                                xg[k - 1][128 * vt : 128 * (vt + 1), :], xe[:]
                            )
                if k == K - 1:
                    out_stage(ch, nvt, (lambda kk: CH[0]) if _out_k0 else (lambda kk: CH[kk]))
        if _skip_rec:
            for ch in range(NVCH):
                out_stage(ch, 4 if ch < 5 else 1, lambda kk: CH[0])
        if chdump is not None:
            for vt in range(NT):
                nc.sync.dma_start(chdump[:, vt, :], CH[_dump_ch][:, vt, :])

    nc.compile()
    return nc


def _host_prep(inputs, lap_rows, lap_cols, lap_vals, weight, bias):
    import ml_dtypes

    inputs = np.asarray(inputs, dtype=np.float32)
    lap_rows = np.asarray(lap_rows)
    lap_cols = np.asarray(lap_cols)
    lap_vals = np.asarray(lap_vals, dtype=np.float32)
    weight = np.asarray(weight, dtype=np.float32)
    bias = np.asarray(bias, dtype=np.float32)

    nnz = lap_rows.shape[0]
    order = np.argsort(lap_rows, kind="stable")
    srows = lap_rows[order]
    assert np.array_equal(
        np.repeat(np.arange(V, dtype=srows.dtype), DEG), srows
    ), "expected exactly DEG entries per row"
    e_cols = np.zeros(EPAD, np.int64)
    e_vals = np.zeros(EPAD, np.float32)
    e_cols[:nnz] = lap_cols[order]
    e_vals[:nnz] = lap_vals[order]

    # per-v-tile dedup: gather each unique col once; SEL folds vals and
    # scatters every (unique col -> output v) pair of the tile
    uniq = [np.unique(e_cols[1024 * vt : 1024 * (vt + 1)]) for vt in range(NT)]
    gchs = tuple(max(1, (len(u) + 127) // 128) for u in uniq)
    toffs = np.concatenate([[0], np.cumsum(gchs)]).astype(np.int64)
    G = int(toffs[NT])
    choffs = tuple(
        int(toffs[min(4 * ch, NT)]) for ch in range(NVCH + 1)
    )

    idx_np = np.zeros((128, G * 8), np.int16)
    sel_np = np.zeros((128, G, 128), np.float32)
    for ch in range(NVCH):
        lo, hi = choffs[ch], choffs[ch + 1]
        gch_ch = hi - lo
        slots = np.zeros(gch_ch * 128, np.int64)
        for vl in range(4 if ch < 5 else 1):
            vt = 4 * ch + vl
            u = uniq[vt]
            base = (toffs[vt] - lo) * 128
            slots[base : base + len(u)] = u
            col2slot = np.zeros(V + 1, np.int64)
            col2slot[u] = np.arange(len(u))
            ecols = e_cols[1024 * vt : 1024 * (vt + 1)]
            evals = e_vals[1024 * vt : 1024 * (vt + 1)]
            sl = col2slot[ecols]  # slot within this tile's groups
            m = np.arange(1024) // DEG  # output row within v-tile
            np.add.at(
                sel_np, (sl % 128, toffs[vt] + sl // 128, m), evals
            )
        # wrapped-16 idx layout over the whole chunk's slot list
        w = slots.reshape(gch_ch * 8, 16).T.astype(np.int16)
        idx_np[:, 8 * lo : 8 * hi] = np.tile(w, (8, 1))

    sel_np = sel_np.astype(ml_dtypes.bfloat16)

    # output-stage weight selection: rows p=s_loc*8+f, cols q=s_loc*8+o
    wsel_np = np.zeros((128, K * 2, 128), np.float32)
    sl = np.arange(16)
    for k in range(K):
        for h in range(2):
            for f in range(FIN):
                for o in range(8):
                    wsel_np[sl * 8 + f, k * 2 + h, sl * 8 + o] = weight[k, f, 8 * h + o]
    wsel_np = wsel_np.astype(ml_dtypes.bfloat16)

    bias_np = np.zeros((128, 2), np.float32)
    p = np.arange(128)
    for h in range(2):
        bias_np[p, h] = bias[8 * h + p % 8]

    # x0 shards: [V, s, f] per core
    xt = inputs.reshape(FIN, V, XYZ).transpose(1, 2, 0)  # [V, 512, 8]
    x0s = []
    for m in range(NCORES):
        x0m = np.zeros((VP, D), np.float32)
        x0m[:V] = xt[:, SLOC * m : SLOC * (m + 1), :].reshape(V, D)
        x0s.append(x0m)
    return x0s, idx_np, sel_np, wsel_np, bias_np, gchs, choffs


def kernel(inputs, lap_rows, lap_cols, lap_vals, weight, bias):
    import ml_dtypes as _ml

    from concourse.bass_utils import run_bass_kernel_spmd

    x0s, idx_np, sel_np, wsel_np, bias_np, gchs, choffs = _host_prep(
        inputs, lap_rows, lap_cols, lap_vals, weight, bias
    )

    key = (gchs, choffs)
    if _COMPILED[0] is None or _COMPILED[0][0] != key:
        _COMPILED[0] = (key, _build(gchs, choffs))
    nc = _COMPILED[0][1]

    in_maps = [
        {
            "x0b": x0s[m].astype(_ml.bfloat16),
            "x0e": x0s[m].astype(_ml.float8_e3m4),
            "idx": idx_np,
            "sel": sel_np,
            "wsel": wsel_np,
            "biasx": bias_np,
        }
        for m in range(NCORES)
    ]
    import os

    trace = bool(int(os.environ.get("KERNEL_TRACE", "0")))
    res = run_bass_kernel_spmd(
        nc, in_maps, core_ids=list(range(NCORES)), trace=trace
    )
    LAST_RESULT[0] = res

    # unshard: outT [8=(t,h), 128=(s_loc,o_loc), VP] per core
    parts = []
    for m in range(NCORES):
        r = res.results[m]["outT"].astype(np.float32)  # [8, 128, VP]
        r = r.reshape(4, 2, 16, 8, VP)[:, :, :, :, :V]  # [t, h, sl, ol, v]
        # o = 8h + ol ; s_local_in_core = 16t + sl
        r = r.transpose(1, 3, 4, 0, 2).reshape(FOUT, V, SLOC)  # [o, v, s]
        parts.append(r)
    out = np.concatenate(parts, axis=2)  # [o, v, 512]
    return np.ascontiguousarray(
        out.reshape(1, FOUT, V, 8, 8, 8).astype(np.float32)
    )


# revision 15
# speedup vs baseline: 1.2741x; 1.1328x over previous
"""Chebyshev graph-conv (gnn_message_passing) Trainium2 kernel.

Reference computation:
    x0 = inputs [1,8,V,8,8,8] -> [V, Fin*B*X*Y*Z]
    Chebyshev recurrence with sparse Laplacian (COO, 8 entries/row), K=5
    out = einsum('kvfbxyz,kfo->bovxyz', cheb, weight) + bias

Sharding: dense dim D = Fin*XYZ split over the XYZ axis across 8 cores
(64 spatial positions per core -> local D = 64*8 = 512, laid out d = s*8+f).

Per-core algorithm (v2):
  - spmv: per-v-tile deduped SWDGE dma_gather of x rows in float8_e3m4
    (halves gather HBM bytes vs bf16; E3M4's 4 mantissa bits keep the
    recurrence error ~1e-2 << 2e-2 budget), folded by PE matmuls with
    bf16 selection matrices into PSUM (8-way segment sum + edge values).
  - All 5 Chebyshev terms stay resident in SBUF (bf16); the combine
    (x_k = 2*psum - x_{k-2}) reads prev from SBUF, writes the bf16
    resident slice + an e3m4 DRAM copy (gather source for the next spmv).
  - Output einsum accumulates over k in PSUM (tensor engine) from
    PE-transposed resident cheb tiles; bias added on drain; out stored
    bf16 and upcast on host.
"""

import sys

for _p in ("/opt/trn_rl_repo", "/root/.axon_site/_ro/trn_rl_repo"):
    if _p not in sys.path:
        sys.path.append(_p)

import numpy as np

V = 2562
DEG = 8
B, FIN, FOUT, K = 1, 8, 16, 5
XYZ = 512
NCORES = 8
SLOC = XYZ // NCORES  # 64 spatial positions per core
D = SLOC * FIN  # 512 local dense dim, d = s_loc*8 + f

VP = 2688  # V padded to 21*128
NT = VP // 128  # 21 v-tiles
EPAD = VP * DEG  # 21504 padded edges
NVCH = 6  # v-chunks of up to 4 v-tiles (5*4 + 1)

_COMPILED = [None]
LAST_RESULT = [None]


def _build(gchs, choffs):
    """gchs[vt] = gathered 128-row groups for v-tile vt; choffs[ch] = group
    offset of chunk ch (choffs[NVCH] = total groups G)."""
    import os as _os
    from contextlib import ExitStack

    import concourse.mybir as mybir
    import concourse.tile as tile
    from concourse import bacc

    _skip_out = bool(int(_os.environ.get("SKIP_OUT", "0")))
    _skip_rec = bool(int(_os.environ.get("SKIP_REC", "0")))
    _xe_from_psv = bool(int(_os.environ.get("XE_FROM_PSV", "0")))
    _split_copy = bool(int(_os.environ.get("SPLIT_COPY", "0")))
    _gather_bf16 = bool(int(_os.environ.get("GATHER_BF16", "0")))
    _single_gather = bool(int(_os.environ.get("SINGLE_GATHER", "1")))
    _out_k0 = bool(int(_os.environ.get("OUT_K0", "0")))
    _dump_ch = int(_os.environ.get("DUMP_CH", "-1"))

    fp32 = mybir.dt.float32
    i16 = mybir.dt.int16
    bf16 = mybir.dt.bfloat16
    f8e3 = mybir.dt.float8e3
    Alu = mybir.AluOpType

    G = choffs[NVCH]
    gdt_is_bf16 = _gather_bf16
    gch_max = max(
        choffs[ch + 1] - choffs[ch] for ch in range(NVCH)
    )

    nc = bacc.Bacc(None, target_bir_lowering=False)

    x0b = nc.dram_tensor("x0b", [VP, D], bf16, kind="ExternalInput")
    gdt = bf16 if _gather_bf16 else f8e3
    x0e = nc.dram_tensor("x0e", [VP, D], gdt, kind="ExternalInput")
    idxd = nc.dram_tensor("idx", [128, G * 8], i16, kind="ExternalInput")
    idxtd = nc.dram_tensor("idxt", [128, NT * 8], i16, kind="ExternalInput")
    seld = nc.dram_tensor("sel", [128, G, 128], bf16, kind="ExternalInput")
    wseld = nc.dram_tensor("wsel", [128, K * 2, 128], bf16, kind="ExternalInput")
    biasd = nc.dram_tensor("biasx", [128, 2], fp32, kind="ExternalInput")
    outd = nc.dram_tensor("outT", [8, 128, VP], bf16, kind="ExternalOutput")
    chdump = (
        nc.dram_tensor("chdump", [128, NT, D], bf16, kind="ExternalOutput")
        if _dump_ch >= 0
        else None
    )

    from concourse.masks import make_identity

    with ExitStack() as ctx:
        tc = ctx.enter_context(tile.TileContext(nc))
        const = ctx.enter_context(tc.tile_pool(name="const", bufs=1))
        chp = ctx.enter_context(tc.tile_pool(name="cheb", bufs=1))
        dram = ctx.enter_context(tc.tile_pool(name="dram", bufs=1, space="DRAM"))
        gp = ctx.enter_context(tc.tile_pool(name="g", bufs=4))
        xgp = ctx.enter_context(tc.tile_pool(name="xtg", bufs=2))
        sp = ctx.enter_context(tc.tile_pool(name="stream", bufs=4))
        xtp = ctx.enter_context(tc.tile_pool(name="xt", bufs=2))
        op = ctx.enter_context(tc.tile_pool(name="ob", bufs=2))
        ppv = ctx.enter_context(tc.tile_pool(name="psumv", bufs=2, space="PSUM"))
        ppo = ctx.enter_context(tc.tile_pool(name="psumo", bufs=1, space="PSUM"))
        ppt = ctx.enter_context(tc.tile_pool(name="psumt", bufs=2, space="PSUM"))

        SEL = const.tile([128, G, 128], bf16)
        WSEL = const.tile([128, K * 2, 128], bf16)
        IDX = const.tile([128, G * 8], i16)
        IDXT = const.tile([128, NT * 8], i16)
        BIA = const.tile([128, 2], fp32)
        IDNB = const.tile([128, 128], bf16)
        IDN = const.tile([128, 128], fp32)
        for q in range(4):
            qs = slice(q * G // 4, (q + 1) * G // 4 if q < 3 else G)
            nc.sync.dma_start(SEL[:, qs, :], seld[:, qs, :])
        nc.sync.dma_start(WSEL[:], wseld[:])
        nc.sync.dma_start(IDX[:], idxd[:])
        nc.sync.dma_start(IDXT[:], idxtd[:])
        nc.sync.dma_start(BIA[:], biasd[:])
        make_identity(nc, IDN[:])
        nc.vector.tensor_copy(IDNB[:], IDN[:])

        # resident Chebyshev terms, bf16 [128, NT*512]
        CH = [
            chp.tile([128, NT, D], bf16, tag=f"ch{k}", name=f"ch{k}")
            for k in range(K)
        ]
        xg = [dram.tile([VP, D], gdt, tag=f"xg{i}", name=f"xg{i}") for i in range(3)]

        def out_stage(ch, nvt, k_src):
            """PSUM-accumulated over k output for chunk ch (nvt v-tiles).

            k=0 arrives pre-transposed via a DMA transpose-mode gather from
            x0b (sequential indices); k=1..4 are PE-transposed from the
            resident cheb terms with one wide PSUM->SBUF copy per (half, k).
            """
            if _skip_out:
                return
            n = nvt * 128
            xTg = xgp.tile([128, 4, n], bf16, tag=f"xTg{n}")
            nc.gpsimd.dma_gather(
                xTg[:, :, :],
                x0b[:],
                IDXT[:, 8 * 4 * ch : 8 * (4 * ch + nvt)],
                num_idxs=n,
                num_idxs_reg=n,
                elem_size=D,
                transpose=True,
                queue_num=0,
            )
            for half in range(2):
                psO = ppo.tile([128, 4, 512], fp32, tag="psO")
                for ti in range(2):
                    for h in range(2):
                        nc.tensor.matmul(
                            psO[:, 2 * ti + h, :n],
                            WSEL[:, h, :],
                            xTg[:, 2 * half + ti, :n],
                            start=True,
                            stop=False,
                        )
                for k in range(1, K):
                    psT = ppt.tile([128, 2, 512], bf16, tag="psT")
                    for vl in range(nvt):
                        vt = 4 * ch + vl
                        for ti in range(2):
                            t = 2 * half + ti
                            nc.tensor.transpose(
                                psT[:, ti, 128 * vl : 128 * (vl + 1)],
                                k_src(k)[:, vt, 128 * t : 128 * (t + 1)],
                                IDNB[:],
                            )
                    xT = xtp.tile([128, 2, 512], bf16, tag="xT")
                    nc.scalar.copy(xT[:, :, :n], psT[:, :, :n])
                    for ti in range(2):
                        for h in range(2):
                            nc.tensor.matmul(
                                psO[:, 2 * ti + h, :n],
                                WSEL[:, k * 2 + h, :],
                                xT[:, ti, :n],
                                start=False,
                                stop=(k == K - 1),
                            )
                ob = op.tile([128, 4, 512], bf16, tag="ob")
                for ti in range(2):
                    for h in range(2):
                        i = (2 * half + ti) * 2 + h
                        nc.vector.tensor_scalar(
                            ob[:, 2 * ti + h, :n],
                            psO[:, 2 * ti + h, :n],
                            BIA[:, h : h + 1],
                            None,
                            Alu.add,
                        )
                        nc.sync.dma_start(
                            outd[i, :, 512 * ch : 512 * ch + n],
                            ob[:, 2 * ti + h, :n],
                        )

        # ---- k = 0: cheb_0 = x0 (straight load into residency) ----
        for vt in range(NT):
            nc.sync.dma_start(CH[0][:, vt, :], x0b[128 * vt : 128 * (vt + 1), :])

        # ---- k = 1..4: x_k = 2 L x_{k-1} - x_{k-2}   (k=1: x_1 = L x_0) ----
        for k in range(1, (0 if _skip_rec else K)):
            src = x0e if k == 1 else xg[k - 2]
            for ch in range(NVCH):
                nvt = 4 if ch < 5 else 1
                gch_ch = choffs[ch + 1] - choffs[ch]
                gchm = max(gchs)
                g_ch = None
                if _single_gather:
                    gts = []
                    goff0 = 0
                    for vl in range(nvt):
                        vt = 4 * ch + vl
                        nidx = gchs[vt] * 128
                        gt = gp.tile([128, gchm, D], gdt, tag="g")
                        with tc.high_priority(offset=400):
                            nc.gpsimd.dma_gather(
                                gt[:, : gchs[vt], :],
                                src[:],
                                IDX[
                                    :,
                                    8 * (choffs[ch] + goff0) : 8
                                    * (choffs[ch] + goff0 + gchs[vt]),
                                ],
                                num_idxs=nidx,
                                num_idxs_reg=nidx,
                                elem_size=D,
                            )
                        gts.append(gt)
                        goff0 += gchs[vt]
                else:
                    g_ch = gp.tile([128, gch_max, D], gdt, tag="gm")
                    with tc.high_priority(offset=400):
                        nc.gpsimd.dma_gather(
                            g_ch[:, :gch_ch, :],
                            src[:],
                            IDX[:, 8 * choffs[ch] : 8 * choffs[ch + 1]],
                            num_idxs=gch_ch * 128,
                            num_idxs_reg=gch_ch * 128,
                            elem_size=D,
                        )
                for vl in range(nvt):
                    vt = 4 * ch + vl
                    goff = sum(gchs[4 * ch + j] for j in range(vl))
                    gsrc = (lambda j: gts[vl][:, j, :]) if _single_gather else (
                        lambda j: g_ch[:, goff + j, :]
                    )
                    psV = ppv.tile([128, 512], fp32, tag="psV")
                    for j in range(gchs[vt]):
                        nc.tensor.matmul(
                            psV[:],
                            SEL[:, choffs[ch] + goff + j, :],
                            gsrc(j),
                            start=(j == 0),
                            stop=(j == gchs[vt] - 1),
                        )
                    with tc.high_priority(offset=600):
                        if k == 1:
                            nc.vector.tensor_copy(CH[1][:, vt, :], psV[:])
                        else:
                            nc.vector.scalar_tensor_tensor(
                                CH[k][:, vt, :],
                                psV[:],
                                2.0,
                                CH[k - 2][:, vt, :],
                                Alu.mult,
                                Alu.subtract,
                            )
                        if k < K - 1:
                            xe = sp.tile([128, D], gdt, tag="xe")
                            if _xe_from_psv:
                                nc.vector.tensor_copy(xe[:], psV[:])
                            else:
                                nc.vector.tensor_copy(xe[:], CH[k][:, vt, :])
                            nc.sync.dma_start(
                                xg[k - 1][128 * vt : 128 * (vt + 1), :], xe[:]
                            )
                if k == K - 1:
                    out_stage(ch, nvt, (lambda kk: CH[0]) if _out_k0 else (lambda kk: CH[kk]))
        if _skip_rec:
            for ch in range(NVCH):
                out_stage(ch, 4 if ch < 5 else 1, lambda kk: CH[0])
        if chdump is not None:
            for vt in range(NT):
                nc.sync.dma_start(chdump[:, vt, :], CH[_dump_ch][:, vt, :])

    nc.compile()
    return nc


def _host_prep(inputs, lap_rows, lap_cols, lap_vals, weight, bias):
    import ml_dtypes

    inputs = np.asarray(inputs, dtype=np.float32)
    lap_rows = np.asarray(lap_rows)
    lap_cols = np.asarray(lap_cols)
    lap_vals = np.asarray(lap_vals, dtype=np.float32)
    weight = np.asarray(weight, dtype=np.float32)
    bias = np.asarray(bias, dtype=np.float32)

    nnz = lap_rows.shape[0]
    order = np.argsort(lap_rows, kind="stable")
    srows = lap_rows[order]
    assert np.array_equal(
        np.repeat(np.arange(V, dtype=srows.dtype), DEG), srows
    ), "expected exactly DEG entries per row"
    e_cols = np.zeros(EPAD, np.int64)
    e_vals = np.zeros(EPAD, np.float32)
    e_cols[:nnz] = lap_cols[order]
    e_vals[:nnz] = lap_vals[order]

    # per-v-tile dedup: gather each unique col once; SEL folds vals and
    # scatters every (unique col -> output v) pair of the tile
    uniq = [np.unique(e_cols[1024 * vt : 1024 * (vt + 1)]) for vt in range(NT)]
    gchs = tuple(max(1, (len(u) + 127) // 128) for u in uniq)
    toffs = np.concatenate([[0], np.cumsum(gchs)]).astype(np.int64)
    G = int(toffs[NT])
    choffs = tuple(
        int(toffs[min(4 * ch, NT)]) for ch in range(NVCH + 1)
    )

    idx_np = np.zeros((128, G * 8), np.int16)
    sel_np = np.zeros((128, G, 128), np.float32)
    for ch in range(NVCH):
        lo, hi = choffs[ch], choffs[ch + 1]
        gch_ch = hi - lo
        slots = np.zeros(gch_ch * 128, np.int64)
        for vl in range(4 if ch < 5 else 1):
            vt = 4 * ch + vl
            u = uniq[vt]
            base = (toffs[vt] - lo) * 128
            slots[base : base + len(u)] = u
            col2slot = np.zeros(V + 1, np.int64)
            col2slot[u] = np.arange(len(u))
            ecols = e_cols[1024 * vt : 1024 * (vt + 1)]
            evals = e_vals[1024 * vt : 1024 * (vt + 1)]
            sl = col2slot[ecols]  # slot within this tile's groups
            m = np.arange(1024) // DEG  # output row within v-tile
            np.add.at(
                sel_np, (sl % 128, toffs[vt] + sl // 128, m), evals
            )
        # wrapped-16 idx layout over the whole chunk's slot list
        w = slots.reshape(gch_ch * 8, 16).T.astype(np.int16)
        idx_np[:, 8 * lo : 8 * hi] = np.tile(w, (8, 1))

    sel_np = sel_np.astype(ml_dtypes.bfloat16)

    # output-stage weight selection: rows p=s_loc*8+f, cols q=s_loc*8+o
    wsel_np = np.zeros((128, K * 2, 128), np.float32)
    sl = np.arange(16)
    for k in range(K):
        for h in range(2):
            for f in range(FIN):
                for o in range(8):
                    wsel_np[sl * 8 + f, k * 2 + h, sl * 8 + o] = weight[k, f, 8 * h + o]
    wsel_np = wsel_np.astype(ml_dtypes.bfloat16)

    bias_np = np.zeros((128, 2), np.float32)
    p = np.arange(128)
    for h in range(2):
        bias_np[p, h] = bias[8 * h + p % 8]

    # transpose-gather sequential indices, wrapped-16
    idxt_np = np.tile(
        np.arange(VP, dtype=np.int16).reshape(VP // 16, 16).T, (8, 1)
    )

    # x0 shards: [V, s, f] per core
    xt = inputs.reshape(FIN, V, XYZ).transpose(1, 2, 0)  # [V, 512, 8]
    x0s = []
    for m in range(NCORES):
        x0m = np.zeros((VP, D), np.float32)
        x0m[:V] = xt[:, SLOC * m : SLOC * (m + 1), :].reshape(V, D)
        x0s.append(x0m)
    return x0s, idx_np, idxt_np, sel_np, wsel_np, bias_np, gchs, choffs


def kernel(inputs, lap_rows, lap_cols, lap_vals, weight, bias):
    import ml_dtypes as _ml

    from concourse.bass_utils import run_bass_kernel_spmd

    x0s, idx_np, idxt_np, sel_np, wsel_np, bias_np, gchs, choffs = _host_prep(
        inputs, lap_rows, lap_cols, lap_vals, weight, bias
    )

    key = (gchs, choffs)
    if _COMPILED[0] is None or _COMPILED[0][0] != key:
        _COMPILED[0] = (key, _build(gchs, choffs))
    nc = _COMPILED[0][1]

    in_maps = [
        {
            "x0b": x0s[m].astype(_ml.bfloat16),
            "x0e": x0s[m].astype(_ml.float8_e3m4),
            "idx": idx_np,
            "idxt": idxt_np,
            "sel": sel_np,
            "wsel": wsel_np,
            "biasx": bias_np,
        }
        for m in range(NCORES)
    ]
    import os

    trace = bool(int(os.environ.get("KERNEL_TRACE", "0")))
    res = run_bass_kernel_spmd(
        nc, in_maps, core_ids=list(range(NCORES)), trace=trace
    )
    LAST_RESULT[0] = res

    # unshard: outT [8=(t,h), 128=(s_loc,o_loc), VP] per core
    parts = []
    for m in range(NCORES):
        r = res.results[m]["outT"].astype(np.float32)  # [8, 128, VP]
        r = r.reshape(4, 2, 16, 8, VP)[:, :, :, :, :V]  # [t, h, sl, ol, v]
        # o = 8h + ol ; s_local_in_core = 16t + sl
        r = r.transpose(1, 3, 4, 0, 2).reshape(FOUT, V, SLOC)  # [o, v, s]
        parts.append(r)
    out = np.concatenate(parts, axis=2)  # [o, v, 512]
    return np.ascontiguousarray(
        out.reshape(1, FOUT, V, 8, 8, 8).astype(np.float32)
    )
